# revision 3
# baseline (speedup 1.0000x reference)
"""Trainium2 Bass kernel for CascadedLoRALinear:
    out = x @ W^T + b + 4.0 * (x @ A^T) @ B^T
        + 2.0 * ((((x @ A1^T) @ A2^T) @ B1^T) @ B2^T)

Sharding: data-parallel over tokens (batch*seq = 8192 -> 1024/core on 8 cores),
all weights replicated.

fp8 variant v3: the dominant x @ W^T contraction runs as fp8e4 DoubleRow
matmuls (virtual 256-row PE, 2 fp8 MACs/cell/cycle, 216 ns per K=256 x
[128 tok x 512 out] step = the fp8 roofline), while the LoRA chain stays fp16
(its lora1 term dominates the output norm, so fp8 error there would be ~5%
rel err; on the base term it dilutes to ~0.9%). x's fp8 copy is packed host-side
(on-device DVE/ACT quantization was tried and slowed the PE ~15% via SBUF
port contention with the matmul operand stream).

v3 structure changes vs v2:
  - coarse DMA: x^T in 8 four-ko group descriptors (was 32), W in 2 half-
    n-tile descriptors per n (was 8), [A;A1]^T in 2 halves (was 8) -- fewer
    sync-queue dispatches and semaphores in the DMA-bound preamble.
  - chain matmuls batched per 4-ko group and early-B DR matmuls batched per
    group: ~16 fp16<->fp8 PE mode switches in the load phase instead of 64.
  - steady state: Rc (rank-correction) fp16 matmuls batched per 4 output
    tiles (accumulation groups on separate PSUM banks stay open across the
    batch), quartering the DR<->fp16 mode switches.
  - finalize copies alternate ACT/DVE so PSUM banks drain at 2x rate.

Scaling scheme (to keep fp8 operands in e4m3's +-240 range and share one PSUM
accumulation group): x_q = 32*x, W_q = 2048*W^T, so the fp8 partial sums carry
65536x the base term. The fp16 rank-correction matmul matches by scaling the
chain 256x (via ATc) and the combined correction matrix Rc 256x. One scaled
PSUM->SBUF copy (x 1/65536) restores units.
"""

import sys

import numpy as np

try:
    import concourse.bass  # noqa: F401
except ImportError:
    sys.path.insert(0, "/opt/trn_rl_repo")

import ml_dtypes
import concourse.mybir as mybir
import concourse.tile as tile
from concourse import bacc
from concourse.bass_utils import run_bass_kernel_spmd

F16 = np.float16
F8 = ml_dtypes.float8_e4m3

N_CORES = 8
NTOK, IN, OUT = 8192, 4096, 4096
TOK = NTOK // N_CORES          # 1024 tokens per core
P = 128
KO = IN // P                   # 32 contraction chunks
KP = KO // 2                   # 16 DoubleRow chunk-pairs
NTILE = 512
NT = OUT // NTILE              # 8 out tiles
TT = TOK // P                  # 8 token tiles
MT = TOK // NTILE              # 2 token macro-tiles (chain phase)
XG = 4                         # ko chunks per x-group DMA
NXG = KO // XG                 # 8 x-groups
KH = KO // 2                   # 16 ko chunks per W half-tile
S1, S2 = 4.0, 2.0
SX, SW = 32.0, 2048.0          # fp8 input scales
SH = 256.0                     # chain / Rc scale (SH*SH == SX*SW)
INV_S = 1.0 / (SX * SW)

_nc_cache = None


def _build():
    nc = bacc.Bacc(None, target_bir_lowering=False)
    bf = mybir.dt.float16
    f8 = mybir.dt.float8e4
    f32 = mybir.dt.float32
    DR = mybir.MatmulPerfMode.DoubleRow

    xT_d = nc.declare_dram_parameter("xT", [P, KO, TOK], bf, isOutput=False)
    xq_d = nc.declare_dram_parameter("xq", [P, NXG, XG, TOK], f8, isOutput=False)
    W_d = nc.declare_dram_parameter("Wq", [NT, P, 2, KH, NTILE], f8, isOutput=False)
    ATc_d = nc.declare_dram_parameter("ATc", [P, 2, KH, 96], bf, isOutput=False)
    A2T_d = nc.declare_dram_parameter("A2T", [P, 64], bf, isOutput=False)
    B1T_d = nc.declare_dram_parameter("B1T", [P, 32], bf, isOutput=False)
    Rc_d = nc.declare_dram_parameter("Rc", [P, OUT], bf, isOutput=False)
    out_d = nc.declare_dram_parameter("out", [TOK, OUT], f32, isOutput=True)

    with tile.TileContext(nc) as tc:
        with (
            tc.tile_pool(name="persist", bufs=1) as persist,
            tc.tile_pool(name="wpool", bufs=4) as wpool,
            tc.tile_pool(name="h3pool", bufs=2) as h3pool,
            tc.tile_pool(name="outpool", bufs=4) as outpool,
            tc.tile_pool(name="psum_out", bufs=5, space="PSUM") as psum_out,
            tc.tile_pool(name="psum_h", bufs=1, space="PSUM") as psum_h,
        ):
            # PE warmup: junk matmuls on an uninitialized scratch tile (the
            # scratch PSUM result is never read), issued before any data
            # lands, so the HAM clock gate opens (1.2 -> 2.4 GHz after
            # ~3.4us of activity) before the real stream starts
            warm_sb = persist.tile([P, P], bf)
            nc.vector.memset(warm_sb[:], 0.0)
            warm_ps = psum_h.tile([P, P], f32, tag="h3", name="warm_ps")
            for _ in range(64):
                nc.tensor.matmul(warm_ps[:], warm_sb[:], warm_sb[:],
                                 start=True, stop=True)

            def w_halves(n):
                halves = []
                for h in range(2):
                    wt = wpool.tile([P, KH, NTILE], f8, tag="w")
                    nc.sync.dma_start(out=wt[:], in_=W_d[n, :, h, :, :])
                    halves.append(wt)
                return halves

            # x^T loaded as 8 fp16 four-ko group tiles (chain inputs); the
            # fp8 copy quantized on-device into 16 ko-pair tiles (DoubleRow
            # lhsT, DVE for the even pair / ACT for the odd); [A;A1]^T in 2
            # halves.  W n=0's two halves interleave so the during-load
            # compute is fed in lockstep.
            xg = [persist.tile([P, XG, TOK], bf, tag=f"x{g}", name=f"xg{g}")
                  for g in range(NXG)]
            xq = [persist.tile([P, XG, TOK], f8, tag=f"xq{g}", name=f"xq{g}")
                  for g in range(NXG)]
            atq = [persist.tile([P, KH, 96], bf, tag=f"at{h}", name=f"atq{h}")
                   for h in range(2)]
            w0 = []

            def load_group(g):
                nc.sync.dma_start(out=xg[g][:], in_=xT_d[:, g * XG:(g + 1) * XG, :])
                nc.sync.dma_start(out=xq[g][:], in_=xq_d[:, g, :, :])

            nc.sync.dma_start(out=atq[0][:], in_=ATc_d[:, 0, :, :])
            load_group(0)
            load_group(1)
            wt = wpool.tile([P, KH, NTILE], f8, tag="w", name="w0_0")
            nc.sync.dma_start(out=wt[:], in_=W_d[0, :, 0, :, :])
            w0.append(wt)
            load_group(2)
            load_group(3)
            nc.sync.dma_start(out=atq[1][:], in_=ATc_d[:, 1, :, :])
            wt = wpool.tile([P, KH, NTILE], f8, tag="w", name="w0_1")
            nc.sync.dma_start(out=wt[:], in_=W_d[0, :, 1, :, :])
            w0.append(wt)
            for g in range(4, NXG):
                load_group(g)

            a2t = persist.tile([P, 64], bf)
            nc.sync.dma_start(out=a2t[:], in_=A2T_d[:])
            b1t = persist.tile([P, 32], bf)
            nc.sync.dma_start(out=b1t[:], in_=B1T_d[:])
            rcs = persist.tile([P, OUT], bf)
            nc.sync.dma_start(out=rcs[:], in_=Rc_d[:])

            hcomb = persist.tile([P, TOK], bf)
            nc.any.memset(hcomb[96:128, :], 0.0)
            nc.any.memset(hcomb[96:97, :], SH)

            def finalize(po, tt, n, ocols, last=False):
                trows = slice(tt * P, (tt + 1) * P)
                rc_mm(po, tt, ocols)
                ot = outpool.tile([P, NTILE], f32)
                if last:
                    # split the final tile ACT/DVE so its DMA starts sooner
                    HB = NTILE // 2
                    nc.scalar.mul(ot[:, 0:HB], po[:, 0:HB], INV_S)
                    nc.vector.tensor_scalar_mul(ot[:, HB:], po[:, HB:], INV_S)
                    oc_a = slice(ocols.start, ocols.start + HB)
                    oc_b = slice(ocols.start + HB, ocols.stop)
                    nc.sync.dma_start(out=out_d[trows, oc_a], in_=ot[:, 0:HB])
                    nc.sync.dma_start(out=out_d[trows, oc_b], in_=ot[:, HB:])
                else:
                    nc.scalar.mul(ot[:], po[:], INV_S)
                    nc.sync.dma_start(out=out_d[trows, ocols], in_=ot[:])

            def rc_mm(po, tt, ocols):
                trows = slice(tt * P, (tt + 1) * P)
                nc.tensor.matmul(po[:], hcomb[:, trows], rcs[:, ocols],
                                 start=False, stop=True)

            def dr_mm(po, c, trows, wsub, start):
                h, p = divmod(c, KP // 2)
                g, j = divmod(c, 2)
                nc.tensor.matmul(
                    po[:], xq[g][:, 2 * j:2 * j + 2, trows],
                    wsub[h][:, 2 * p:2 * p + 2, :],
                    start=start, stop=False, perf_mode=DR,
                )

            # during-load phase: rank chain batched per 4-ko group,
            # interleaved with the first 4 token-tiles of n=0 (one group
            # behind), to keep the PE dense while x^T / W stream in
            NEARLY = 4
            h12 = [psum_h.tile([P, NTILE], f32, tag=f"h12_{mt}", name=f"h12_{mt}")
                   for mt in range(MT)]
            po_early = [psum_out.tile([P, NTILE], f32, tag="po", name=f"poe{tt}")
                        for tt in range(NEARLY)]
            oc0 = slice(0, NTILE)

            for g in range(NXG):
                for ko in range(g * XG, (g + 1) * XG):
                    for mt in range(MT):
                        cols = slice(mt * NTILE, (mt + 1) * NTILE)
                        nc.tensor.matmul(
                            h12[mt][0:96, :], atq[ko // KH][:, ko % KH, :],
                            xg[ko // XG][:, ko % XG, cols],
                            start=(ko == 0), stop=(ko == KO - 1),
                        )
                if g >= 1:
                    # early-B DR matmuls for the previous group's 2 ko-pairs
                    for c in (2 * (g - 1), 2 * (g - 1) + 1):
                        for tt in range(NEARLY):
                            trows = slice(tt * P, (tt + 1) * P)
                            dr_mm(po_early[tt], c, trows, w0, start=(c == 0))
                else:
                    # dependency-free filler keeps the PE hot while the next
                    # x group streams in (early-B hasn't started yet)
                    for _ in range(10):
                        nc.tensor.matmul(warm_ps[:], warm_sb[:], warm_sb[:],
                                         start=True, stop=True)
            for c in (2 * (NXG - 1), 2 * (NXG - 1) + 1):
                for tt in range(NEARLY):
                    trows = slice(tt * P, (tt + 1) * P)
                    dr_mm(po_early[tt], c, trows, w0, start=False)

            def base_chunk(po, tt, wsub, c0, c1):
                trows = slice(tt * P, (tt + 1) * P)
                for c in range(c0, c1):
                    dr_mm(po, c, trows, wsub, start=(c == 0))

            # chain tail: cascade + hcomb assembly, interleaved with tt4's
            # DR chunks (inputs resident) so the cascade's DVE copy
            # latencies hide under short DR bursts instead of stalling the
            # PE; same per-mt write/read order as the plain version
            po4 = psum_out.tile([P, NTILE], f32, tag="po")
            t4 = NEARLY
            cA = slice(0, NTILE)
            cB = slice(NTILE, 2 * NTILE)
            nc.vector.tensor_copy(out=hcomb[0:96, cA], in_=h12[0][0:96, :])
            base_chunk(po4, t4, w0, 0, 4)
            h3p0 = psum_h.tile([P, NTILE], f32, tag="h3", name="h3p0")
            nc.tensor.matmul(h3p0[0:64, :], a2t[:], hcomb[:, cA],
                             start=True, stop=True)
            h3s0 = h3pool.tile([P, NTILE], bf)
            nc.any.memset(h3s0[64:128, :], 0.0)
            nc.vector.tensor_copy(out=h3s0[0:64, :], in_=h3p0[0:64, :])
            base_chunk(po4, t4, w0, 4, 8)
            h4p0 = psum_h.tile([P, NTILE], f32, tag="h3", name="h4p0")
            nc.tensor.matmul(h4p0[64:96, :], b1t[:], h3s0[:],
                             start=True, stop=True)
            nc.vector.tensor_copy(out=hcomb[64:96, cA], in_=h4p0[64:96, :])
            nc.vector.tensor_copy(out=hcomb[0:96, cB], in_=h12[1][0:96, :])
            base_chunk(po4, t4, w0, 8, 12)
            h3p1 = psum_h.tile([P, NTILE], f32, tag="h3", name="h3p1")
            nc.tensor.matmul(h3p1[0:64, :], a2t[:], hcomb[:, cB],
                             start=True, stop=True)
            h3s1 = h3pool.tile([P, NTILE], bf)
            nc.any.memset(h3s1[64:128, :], 0.0)
            nc.vector.tensor_copy(out=h3s1[0:64, :], in_=h3p1[0:64, :])
            base_chunk(po4, t4, w0, 12, 14)
            h4p1 = psum_h.tile([P, NTILE], f32, tag="h3", name="h4p1")
            nc.tensor.matmul(h4p1[64:96, :], b1t[:], h3s1[:],
                             start=True, stop=True)
            base_chunk(po4, t4, w0, 14, 16)
            nc.vector.tensor_copy(out=hcomb[64:96, cB], in_=h4p1[64:96, :])
            for tt in range(NEARLY):
                finalize(po_early[tt], tt, 0, oc0)
            finalize(po4, t4, 0, oc0)

            # phase B: remaining tiles, per-tile Rc + finalize (short fp8
            # bursts keep the HAM DVFS state at full clock)
            w_next = w_halves(1)
            cur_n = 0
            wsub = w0
            todo = [(0, tt) for tt in range(NEARLY + 1, TT)]
            todo += [(n, tt) for n in range(1, NT) for tt in range(TT)]
            for n, tt in todo:
                if n != cur_n:
                    wsub = w_next
                    cur_n = n
                    if n + 1 < NT:
                        w_next = w_halves(n + 1)
                ocols = slice(n * NTILE, (n + 1) * NTILE)
                po = psum_out.tile([P, NTILE], f32, tag="po")
                base_chunk(po, tt, wsub, 0, KP)
                finalize(po, tt, n, ocols,
                         last=(n == NT - 1 and tt == TT - 1))
    nc.compile()
    return nc


def _get_nc():
    global _nc_cache
    if _nc_cache is None:
        _nc_cache = _build()
    return _nc_cache


def make_in_maps(x, W, b, A, B, A1, A2, B1, B2):
    """Host-side shard + pack. Returns per-core in_maps for run_bass_kernel_spmd."""
    x = np.ascontiguousarray(np.asarray(x, np.float32)).reshape(NTOK, IN)
    W = np.asarray(W, np.float32)
    b = np.asarray(b, np.float32)
    A = np.asarray(A, np.float32)
    B = np.asarray(B, np.float32)
    A1 = np.asarray(A1, np.float32)
    A2 = np.asarray(A2, np.float32)
    B1 = np.asarray(B1, np.float32)
    B2 = np.asarray(B2, np.float32)

    # W^T [IN, OUT] -> [NT, P, 2, KH, NTILE] fp8 so each half-n-tile DMA is
    # one contiguous descriptor
    Wq = np.ascontiguousarray(
        np.clip(W.T * SW, -240, 240).astype(F8)
        .reshape(KO, P, NT, NTILE).transpose(2, 1, 0, 3)
        .reshape(NT, P, 2, KH, NTILE)
    )
    ATc = np.ascontiguousarray(
        (SH * np.concatenate([A.T, A1.T], axis=1)).astype(F16)
        .reshape(KO, P, 96).transpose(1, 0, 2).reshape(P, 2, KH, 96)
    )
    A2T = np.zeros((P, 64), F16)
    A2T[64:96] = A2.T.astype(F16)
    B1T = np.zeros((P, 32), F16)
    B1T[0:64] = B1.T.astype(F16)
    Rc = np.zeros((P, OUT), F16)
    Rc[0:64] = (S1 * SH * B.T).astype(F16)
    Rc[64:96] = (S2 * SH * B2.T).astype(F16)
    Rc[96] = (SH * b).astype(F16)

    in_maps = []
    for c in range(N_CORES):
        xs = x[c * TOK:(c + 1) * TOK]                      # [TOK, IN]
        xsT = xs.T.reshape(KO, P, TOK)
        xT = np.ascontiguousarray(xsT.astype(F16).transpose(1, 0, 2))
        xq = np.ascontiguousarray(
            np.clip(xsT * SX, -240, 240).astype(F8).transpose(1, 0, 2)
            .reshape(P, NXG, XG, TOK)
        )
        in_maps.append(
            {"xT": xT, "xq": xq, "Wq": Wq, "ATc": ATc, "A2T": A2T,
             "B1T": B1T, "Rc": Rc}
        )
    return in_maps


def kernel(x, W, b, A, B, A1, A2, B1, B2):
    nc = _get_nc()
    in_maps = make_in_maps(x, W, b, A, B, A1, A2, B1, B2)
    res = run_bass_kernel_spmd(nc, in_maps, core_ids=list(range(N_CORES)))
    out = np.concatenate([res.results[c]["out"] for c in range(N_CORES)], axis=0)
    return out.reshape(4, 2048, OUT)
